# revision 1
# baseline (speedup 1.0000x reference)
"""ConvNMF loss kernel for Trainium2, sharded over 8 NeuronCores.

Math (see reference):
    W = softplus(W_pre)            # (F, K, L)
    H = softplus(H_pre)            # (K, T + L - 1)
    pred[f, t] = sum_{k,l} W[f,k,l] * H[k, T-1+l-t]
    out = sum((pred - data)^2) / (F*T)

Sharding: timebins split across 8 cores (sequence parallel), W replicated,
each core's H shard carries an L-1 halo. Per-core partial SSE is returned as
a [128,1] per-partition vector; the host does the final (tiny) reduction.

Device-side formulation: with Hs[k,j] = softplus(H_pre[k, T+30-t0-j]) (the
host flips each H shard in time), the core computes the forward correlation
    pred[f, t0+i] = sum_{k,l'} W[f,k,31-l'] * Hs[k, i+l']
as 16 accumulating 128-contraction matmuls per output tile: lag pairs
(2j, 2j+1) are packed into the 128 partitions (64 components x 2 lags). The
packed rhs Hdup[0:64,m]=Hs[:,m], Hdup[64:128,m]=Hs[:,m+1] is built host-side
from H_pre (pure reindexing; softplus runs on device).
"""
import numpy as np
from contextlib import ExitStack

import concourse.bass as bass
import concourse.bacc as bacc
import concourse.mybir as mybir
import concourse.tile as tile
from concourse import bass_utils

F32 = mybir.dt.float32
F32R = mybir.dt.float32r
AFT = mybir.ActivationFunctionType

F = 256           # n_features
K = 64            # n_components
L = 32            # n_lags
T = 20000         # n_timebins
NCORES = 8
TC = T // NCORES  # timebins per core (2500)
TW = 500          # output tile width (<=512 fp32 PSUM bank, >=256 for f32r rate)
NT = TC // TW     # 5 tiles per f-chunk
NJ = L // 2       # 16 lag pairs -> 16 matmuls per output tile
FCH = F // 128    # 2 f-chunks of 128
HC = TC + L - 1   # 2531 H columns per core (halo included)
NGROUPS = FCH * NT  # 10 PSUM tiles


FP8 = mybir.dt.float8e4
HPAD = 2544      # hq8 row stride, multiple of 16 (DoubleRow AP constraint)
NQ = L // 4      # 8 quad-lag DoubleRow matmuls per output tile


def build_nc(mode="fp8", reps=1):
    nc = bacc.Bacc("TRN2", target_bir_lowering=False, debug=False)
    hdup_d = nc.dram_tensor("hdup", [128, HC], F32, kind="ExternalInput").ap()
    wmat_d = nc.dram_tensor("wmat", [128, NJ * F], F32, kind="ExternalInput").ap()
    data_d = nc.dram_tensor("dat", [F, TC], F32, kind="ExternalInput").ap()
    out_d = nc.dram_tensor("out", [128, NGROUPS], F32, kind="ExternalOutput").ap()

    mmdt = F32R if mode == "f32r" else F32

    with tile.TileContext(nc) as tc, ExitStack() as ctx:
        cpool = ctx.enter_context(tc.tile_pool(name="cpool", bufs=1))
        dpool = ctx.enter_context(tc.tile_pool(name="dpool", bufs=NGROUPS))
        spool = ctx.enter_context(tc.tile_pool(name="spool", bufs=NGROUPS))
        ppool = ctx.enter_context(tc.tile_pool(name="ppool", bufs=4, space="PSUM"))

        # ---- H: chunked DMA -> exp -> ln(x+1) so the first matmuls can
        # start as soon as the first column chunk of softplus(H) lands ----
        HCH = 640
        hraw = cpool.tile([128, HC], F32, tag="hraw")
        hexp = cpool.tile([128, HC], F32, tag="hexp")
        hsp = cpool.tile([128, HC], mmdt, tag="hsp")
        h_edges = list(range(0, HC, HCH)) + [HC]
        # W is consumed interleaved with the first output tile's matmuls;
        # chunk it the same way (4 chunks of 4 lag pairs).
        WCH = 4
        wchunks = []
        for wc in range(NJ // WCH):
            wraw = cpool.tile([128, WCH * F], F32, tag=f"wraw{wc}",
                              name=f"wraw{wc}")
            wexp = cpool.tile([128, WCH * F], F32, tag=f"wexp{wc}",
                              name=f"wexp{wc}")
            w_t = cpool.tile([128, WCH * F], mmdt, tag=f"wsp{wc}",
                             name=f"wsp{wc}")
            wchunks.append((wraw, wexp, w_t))

        # interleave: H chunk 0, W chunk 0, H chunk 1, W chunks 1-3, rest of H
        def emit_h_chunk(i):
            lo, hi = h_edges[i], h_edges[i + 1]
            nc.sync.dma_start(hraw[:, lo:hi], hdup_d[:, lo:hi])
            nc.scalar.activation(hexp[:, lo:hi], hraw[:, lo:hi], AFT.Exp)
            nc.scalar.activation(hsp[:, lo:hi], hexp[:, lo:hi], AFT.Ln,
                                 bias=1.0)

        def emit_w_chunk(wc):
            wraw, wexp, w_t = wchunks[wc]
            nc.sync.dma_start(wraw[:], wmat_d[:, wc * WCH * F:(wc + 1) * WCH * F])
            nc.scalar.activation(wexp[:], wraw[:], AFT.Exp)
            nc.scalar.activation(w_t[:], wexp[:], AFT.Ln, bias=1.0)

        # fp8 path: DoubleRow matmuls contract 256 rows (2 subtiles); H and
        # W are converted f32->fp8e4 on DVE right after each softplus chunk.
        if mode == "fp8":
            hq8 = cpool.tile([128, 2, HPAD], FP8, tag="hq8")
            wq8 = [cpool.tile([128, WCH * F], FP8, tag=f"wq8{wc}",
                              name=f"wq8{wc}")
                   for wc in range(NJ // WCH)]

        def emit_h8_chunk(i):
            lo, hi = h_edges[i], h_edges[i + 1]
            nc.vector.tensor_copy(hq8[:, 0, lo:hi], hsp[:, lo:hi])
            m0 = max(0, lo - 2)
            nc.vector.tensor_copy(hq8[:, 1, m0:hi - 2], hsp[:, m0 + 2:hi])

        def emit_w8_chunk(wc):
            nc.vector.tensor_copy(wq8[wc][:], wchunks[wc][2][:])

        def post_h(i):
            if mode == "fp8":
                emit_h8_chunk(i)

        def post_w(wc):
            if mode == "fp8":
                emit_w8_chunk(wc)

        emit_h_chunk(0); post_h(0)
        emit_w_chunk(0); post_w(0)
        emit_h_chunk(1); post_h(1)
        emit_w_chunk(1); post_w(1)
        emit_w_chunk(2); post_w(2)
        emit_w_chunk(3); post_w(3)
        for i in range(2, len(h_edges) - 1):
            emit_h_chunk(i); post_h(i)

        def w_lhsT(j, c):
            w_t = wchunks[j // WCH][2]
            off = (j % WCH) * F + c * 128
            return w_t[:, off:off + 128]

        # ---- main loop: 10 output tiles ----
        apool = ctx.enter_context(tc.tile_pool(name="apool", bufs=2))
        acc0 = None
        for r in range(reps):
            acc = apool.tile([128, NGROUPS], F32, tag="acc", name=f"acc{r}")
            if r == 0:
                acc0 = acc
            g = 0
            for c in range(FCH):
                for i0 in range(NT):
                    pt = ppool.tile([128, TW], F32, tag="pt", name=f"pt{r}_{g}")
                    base = i0 * TW
                    if mode == "fp8":
                        for jq in range(NQ):
                            wc, q = jq // 2, jq % 2
                            lhsT = wq8[wc][:, q * 512:(q + 1) * 512].rearrange(
                                "p (i m) -> p i m", i=2)[:, :, c * 128:(c + 1) * 128]
                            rhs = hq8[:, :, base + 4 * jq: base + 4 * jq + TW]
                            nc.tensor.matmul(
                                pt[:], lhsT, rhs, start=(jq == 0),
                                stop=(jq == NQ - 1),
                                perf_mode=mybir.MatmulPerfMode.DoubleRow)
                    else:
                        for j in range(NJ):
                            rhs = hsp[:, base + 2 * j: base + 2 * j + TW]
                            nc.tensor.matmul(pt[:], w_lhsT(j, c), rhs,
                                             start=(j == 0), stop=(j == NJ - 1))

                    dt_ = dpool.tile([128, TW], F32, tag="dt", name=f"dt{r}_{g}")
                    nc.sync.dma_start(
                        dt_[:], data_d[c * 128:(c + 1) * 128, base:base + TW])
                    resid = spool.tile([128, TW], F32, tag="resid",
                                       name=f"resid{r}_{g}")
                    nc.vector.tensor_sub(resid[:], pt[:], dt_[:])
                    sq = spool.tile([128, TW], F32, tag="sq", name=f"sq{r}_{g}")
                    nc.scalar.activation(sq[:], resid[:], AFT.Square,
                                         accum_out=acc[:, g:g + 1])
                    g += 1

        # ---- final: DMA the per-tile column sums out; host sums them ----
        nc.sync.dma_start(out_d[:], acc0[:])
    nc.compile()
    return nc


def make_in_maps(data, W_pre, H_pre):
    """Pure host-side resharding/reindexing (no math beyond indexing)."""
    data = np.ascontiguousarray(data, dtype=np.float32)
    W_pre = np.asarray(W_pre, dtype=np.float32)
    H_pre = np.asarray(H_pre, dtype=np.float32)

    # W: [128, 16*256]; rows (l2*64+k), col block j holds lag pair (2j, 2j+1)
    # wmat[l2*64+k, j*256+f] = W_pre[f, k, 31-(2j+l2)]
    wt = np.transpose(W_pre, (2, 1, 0))          # [L, K, F], wt[l,k,f]
    wmat = np.empty((128, NJ * F), dtype=np.float32)
    for j in range(NJ):
        for l2 in range(2):
            wmat[l2 * K:(l2 + 1) * K, j * F:(j + 1) * F] = wt[31 - (2 * j + l2)]

    in_maps = []
    for c in range(NCORES):
        t0 = c * TC
        # Hs_pre[k, j] = H_pre[k, T+30-t0-j], j in [0, HC)
        hrs = H_pre[:, T - TC - t0: T + L - 1 - t0][:, ::-1]  # [K, HC]
        hdup = np.empty((128, HC), dtype=np.float32)
        hdup[:K] = hrs
        hdup[K:, :HC - 1] = hrs[:, 1:]
        hdup[K:, HC - 1] = hrs[:, HC - 1]  # pad col, never read by matmuls
        in_maps.append({
            "hdup": np.ascontiguousarray(hdup),
            "wmat": wmat,
            "dat": np.ascontiguousarray(data[:, t0:t0 + TC]),
        })
    return in_maps


_CACHED_NC = {}


def run_cores(data, W_pre, H_pre, mode="fp8", trace=False):
    if mode not in _CACHED_NC:
        _CACHED_NC[mode] = build_nc(mode)
    nc = _CACHED_NC[mode]
    in_maps = make_in_maps(data, W_pre, H_pre)
    res = bass_utils.run_bass_kernel_spmd(
        nc, in_maps, core_ids=list(range(NCORES)), trace=trace)
    return res


def kernel(data, W_pre, H_pre):
    res = run_cores(data, W_pre, H_pre, mode="f32r", trace=False)
    sse = np.float64(0.0)
    for r in res.results:
        sse += r["out"].astype(np.float64).sum()
    return np.float32(sse / (F * T))



# revision 20
# speedup vs baseline: 2.8125x; 2.8125x over previous
"""ConvNMF loss kernel for Trainium2, sharded over 8 NeuronCores.

Math (see reference):
    W = softplus(W_pre)            # (F, K, L)
    H = softplus(H_pre)            # (K, T + L - 1)
    pred[f, t] = sum_{k,l} W[f,k,l] * H[k, T-1+l-t]
    out = sum((pred - data)^2) / (F*T)

Sharding: timebins split across 8 cores (sequence parallel), W replicated,
each core's H shard carries an L-1 halo. Per-core partial SSE is returned as
a [128, NGROUPS] per-partition/per-tile vector; the host does the final
(tiny) reduction.

Device-side formulation: with Hs[k,j] = softplus(H_pre[k, T+30-t0-j]) (the
host flips each H shard in time), the core computes the forward correlation
    pred[f, t0+u] = sum_{k,l'} W[f,k,31-l'] * Hs[k, u+l']
as 8 accumulating fp8 DoubleRow matmuls (256-deep contraction) per output
tile. Lag packing: partitions carry (l2, k) with l2 in {0,1} the low lag bit
(h8 rows 64:128 hold Hs shifted by one timebin), and the DoubleRow subtile
dimension carries lag bit 2 (shift 2i) via an overlapping stride-2 access
pattern on the flat fp8 H buffer h8, so no second shifted copy of H is
materialized.

H reaches the scalar engine PACKED: hpk[0:64, 318k:318(k+1)] = Hs cols
[636k, 636k+318), hpk[64:128, ...] = Hs cols [636k+318, 636(k+1)) -- the
636-col chunk k occupies both partition halves, halving scalar softplus
columns. DVE (otherwise idle in the first half) unpacks the fp8 result into
h8 and builds the shift-by-1 duplicate rows 64:128.

Engine budget per core (cost-model ns):
  SP     : H-packed (0.65MB) + W (2MB) chunked DMA + out        ~8.5us
  Scalar : one act-table load (natural_log_exp_and_others covers
           Exp/Ln), chunked exp -> ln(1+x) softplus, fp8 out    ~10.7us
  Pool   : all data-tile DMAs (2.56MB, parallel SWDGE queue),
           then per-tile resid = psum_pred - data               ~12.0us
  PE     : 80 DoubleRow matmuls of 500 cols                      ~8.7us
  DVE    : h8 unpack/dup copies + per-tile sum(resid^2)          ~9.5us
"""
import numpy as np
from contextlib import ExitStack

import concourse.bass as bass
import concourse.bacc as bacc
import concourse.mybir as mybir
import concourse.tile as tile
from concourse import bass_utils
from concourse.alu_op_type import AluOpType
from concourse.hw_specs import get_activation_tables

F32 = mybir.dt.float32
FP8 = mybir.dt.float8e4
AFT = mybir.ActivationFunctionType

F = 256           # n_features
K = 64            # n_components
L = 32            # n_lags
T = 20000         # n_timebins
NCORES = 8
TC = T // NCORES  # timebins per core (2500)
TW = 500          # output tile width (<=512 fp32 PSUM bank)
NT = TC // TW     # 5 tiles per f-chunk
NQ = L // 4       # 8 quad-lag DoubleRow matmuls per output tile
FCH = F // 128    # 2 f-chunks of 128
HC = TC + L - 1   # 2531 H columns per core (halo included)
NGROUPS = FCH * NT  # 10 output tiles
WCOLS = (L // 2) * F  # 4096 flat W columns

# H scalar layout: cols [0, 2031) go through a PACKED buffer (two ~508-col
# halves per chunk stacked across the 128 partitions, halving scalar
# columns); cols [2030, 2531) arrive in DIRECT dup layout (rows 64:128 =
# shift-by-1, host-prepared) so the last chunk needs no DVE unpacking on
# the critical end chain. Chunk boundaries align to output tiles.
H_PK_EDGES = [0, 532, 1532, 2531]  # packed chunk edges in H cols
H_PK_HALVES = [(266, 266), (500, 500), (500, 499)]
H_PK_OFF = [0, 266, 766]           # packed col offset per chunk
PKC = 1266                         # total packed cols


def _overlap_rhs(h8, lo, tw):
    """rhs[p, i, m] = h8[p, lo + 2*i + m]: [128, 2, tw] overlapping AP."""
    av = h8[:, lo:lo + tw + 2].unsqueeze(1).copy()
    apl = av.ap.to_list()
    apl[1] = [2, 2]
    apl[2] = [1, tw]
    av.ap.clear()
    for p in apl:
        av.ap.append(p)
    return av


def build_nc(mode="fp8", reps=1):
    nc = bacc.Bacc("TRN2", target_bir_lowering=False, debug=False)
    hpk_d = nc.dram_tensor("hpk", [128, PKC], F32, kind="ExternalInput").ap()
    wmat_d = nc.dram_tensor("wmat", [128, WCOLS], F32, kind="ExternalInput").ap()
    nih_d = nc.dram_tensor("nih", [128, 128], F32, kind="ExternalInput").ap()
    data_d = nc.dram_tensor("dat", [F, TC], F32, kind="ExternalInput").ap()
    out_d = nc.dram_tensor("out", [128, NGROUPS * 6], F32,
                           kind="ExternalOutput").ap()

    with tile.TileContext(nc) as tc, ExitStack() as ctx:
        cpool = ctx.enter_context(tc.tile_pool(name="cpool", bufs=1))
        epool = ctx.enter_context(tc.tile_pool(name="epool", bufs=1))
        dpool = ctx.enter_context(tc.tile_pool(name="dpool", bufs=NGROUPS))
        spool = ctx.enter_context(tc.tile_pool(name="spool", bufs=4))
        apool = ctx.enter_context(tc.tile_pool(name="apool", bufs=2))
        ppool = ctx.enter_context(tc.tile_pool(name="ppool", bufs=8, space="PSUM"))

        # One activation table covers Exp and Ln: load it once up front so
        # the per-function greedy loads never fire.
        tabs = list(get_activation_tables(nc.m.arch).keys())
        nc.scalar.add_instruction(mybir.InstLoadActFuncSet(
            name=nc.get_next_instruction_name(),
            act_func_set_id=tabs.index("natural_log_exp_and_others")))

        hraw = cpool.tile([128, PKC], F32, tag="hraw")
        hpk8 = cpool.tile([128, PKC], FP8, tag="hpk8")
        h8 = cpool.tile([128, HC + 1], FP8, tag="h8")
        wraw = cpool.tile([128, WCOLS], F32, tag="wraw")
        w8 = cpool.tile([128, WCOLS], FP8, tag="w8")
        nihr = cpool.tile([128, 128], F32, tag="nihr")
        nih8 = cpool.tile([128, 128], FP8, tag="nih8")
        nc.gpsimd.dma_start(nihr[:], nih_d)
        nc.vector.tensor_copy(nih8[:], nihr[:])

        w_edges = [0, 256, 1024, 2048, 2816, 3584, 3840, WCOLS]

        def emit_w_chunk(k):
            lo, hi = w_edges[k], w_edges[k + 1]
            nc.sync.dma_start(wraw[:, lo:hi], wmat_d[:, lo:hi])
            wexp = epool.tile([128, hi - lo], F32, tag="wexp", name=f"wexp{k}")
            nc.scalar.activation(wexp[:], wraw[:, lo:hi], AFT.Exp)
            nc.scalar.activation(w8[:, lo:hi], wexp[:], AFT.Ln, bias=1.0)

        def emit_h_chunk(k):
            A, B = H_PK_EDGES[k], H_PK_EDGES[k + 1]
            w0, w1 = H_PK_HALVES[k]
            a = H_PK_OFF[k]
            b = a + w0
            nc.sync.dma_start(hraw[:, a:b], hpk_d[:, a:b])
            hexp = epool.tile([128, w0], F32, tag="hexp", name=f"hexp{k}")
            nc.scalar.activation(hexp[:], hraw[:, a:b], AFT.Exp)
            nc.scalar.activation(hpk8[:, a:b], hexp[:], AFT.Ln, bias=1.0)
            # unpack the two halves into the flat (shift 0) rows of h8
            nc.vector.tensor_copy(h8[0:64, A:A + w0], hpk8[0:64, a:b])
            nc.vector.tensor_copy(h8[0:64, A + w0:B], hpk8[64:128, a:a + w1])
            # shift-by-1 duplicate rows: h8[64:128, j] = h8[0:64, j+1]
            j0 = max(0, A - 1)
            nc.vector.tensor_copy(h8[64:128, j0:B - 1], h8[0:64, j0 + 1:B])

        emit_h_chunk(0)
        emit_w_chunk(0)
        emit_h_chunk(1)
        emit_h_chunk(2)
        emit_w_chunk(1)
        emit_w_chunk(2)
        emit_w_chunk(3)
        emit_w_chunk(4)
        emit_w_chunk(5)
        emit_w_chunk(6)

        def w_lhsT(jq, c):
            B = (c * NQ + jq) * 256
            return w8[:, B:B + 256].rearrange("p (i m) -> p i m", i=2)

        acc0 = None
        for r in range(reps):
            acc = apool.tile([128, NGROUPS, 6], F32, tag="acc", name=f"acc{r}")
            if r == 0:
                acc0 = acc
            # data tiles: all DMAs first on the Pool SWDGE queue (parallel
            # to the SP HWDGE queue carrying H/W)
            dts = []
            for c in range(FCH):
                for t in range(NT):
                    g = c * NT + t   # completion order == acc column order
                    dt_ = dpool.tile([128, TW], F32, tag="dt", name=f"dt{r}_{g}")
                    nc.gpsimd.dma_start(
                        dt_[:], data_d[c * 128:(c + 1) * 128,
                                       t * TW:(t + 1) * TW])
                    dt8 = dpool.tile([128, TW], FP8, tag="dt8",
                                     name=f"dt8{r}_{g}")
                    nc.gpsimd.tensor_copy(dt8[:], dt_[:])
                    dts.append((g, c, t, dt8))
            for (g, c, t, dt8) in dts:
                base = t * TW
                pt = ppool.tile([128, TW], F32, tag="pt", name=f"pt{r}_{g}")
                for jq in range(NQ):
                    rhs = _overlap_rhs(h8, base + 4 * jq, TW)
                    nc.tensor.matmul(
                        pt[:], w_lhsT(jq, c), rhs, start=(jq == 0),
                        stop=False,
                        perf_mode=mybir.MatmulPerfMode.DoubleRow)
                # resid = pred - data, accumulated in PSUM via -I @ data
                nc.tensor.matmul(pt[:], nih8[:], dt8[:], start=False,
                                 stop=True)
                # one-pass per-partition stats of resid (single PSUM input);
                # host reconstructs sum(resid^2) = cvar + count*mean^2
                nc.vector.bn_stats(acc[:, g, :], pt[:])
                if r == reps - 1 and g == NGROUPS - 3:
                    # most acc columns are final: fly them out early (SP
                    # HWDGE -- cheaper completion path than Pool's SWDGE)
                    nc.sync.dma_start(out_d[:, 0:(NGROUPS - 2) * 6],
                                      acc0[:, 0:NGROUPS - 2, :])

        nc.sync.dma_start(out_d[:, (NGROUPS - 2) * 6:],
                          acc0[:, NGROUPS - 2:, :])
    nc.compile()
    return nc


def make_in_maps(data, W_pre, H_pre):
    """Pure host-side resharding/reindexing (no math beyond indexing)."""
    data = np.ascontiguousarray(data, dtype=np.float32)
    W_pre = np.asarray(W_pre, dtype=np.float32)
    H_pre = np.asarray(H_pre, dtype=np.float32)

    # W: [128, 4096]; rows (l2*64+k); cols c-major 128-blocks so the c=0
    # half of the softplus stream is contiguous:
    # wmat[l2*64+k, ((c*8+jq)*2+i)*128 + o] = W_pre[c*128+o, k, 31-(4jq+2i+l2)]
    wt = np.transpose(W_pre, (2, 1, 0))          # [L, K, F], wt[l,k,f]
    wmat = np.empty((128, WCOLS), dtype=np.float32)
    for c in range(FCH):
        for jq in range(NQ):
            for i in range(2):
                B = ((c * NQ + jq) * 2 + i) * 128
                for l2 in range(2):
                    lag = 31 - (4 * jq + 2 * i + l2)
                    wmat[l2 * K:(l2 + 1) * K, B:B + 128] = \
                        wt[lag][:, c * 128:(c + 1) * 128]

    in_maps = []
    for c in range(NCORES):
        t0 = c * TC
        # Hs[k, j] = H_pre[k, T+30-t0-j], j in [0, HC)
        hrs = H_pre[:, T - TC - t0: T + L - 1 - t0][:, ::-1]  # [K, HC]
        # packed chunks: chunk k's two halves [A, A+w0) and [A+w0, B)
        # stacked across the partition dim
        hpk = np.zeros((128, PKC), dtype=np.float32)
        for k in range(len(H_PK_EDGES) - 1):
            A, B = H_PK_EDGES[k], H_PK_EDGES[k + 1]
            w0, w1 = H_PK_HALVES[k]
            a = H_PK_OFF[k]
            hpk[:K, a:a + w0] = hrs[:, A:A + w0]
            hpk[K:, a:a + w1] = hrs[:, A + w0:B]
        nih = np.ascontiguousarray(-np.eye(128, dtype=np.float32))
        in_maps.append({
            "hpk": np.ascontiguousarray(hpk),
            "nih": nih,
            "wmat": wmat,
            "dat": np.ascontiguousarray(data[:, t0:t0 + TC]),
        })
    return in_maps


_CACHED_NC = {}


def run_cores(data, W_pre, H_pre, mode="fp8", trace=False, reps=1):
    key = (mode, reps)
    if key not in _CACHED_NC:
        _CACHED_NC[key] = build_nc(mode, reps=reps)
    nc = _CACHED_NC[key]
    in_maps = make_in_maps(data, W_pre, H_pre)
    res = bass_utils.run_bass_kernel_spmd(
        nc, in_maps, core_ids=list(range(NCORES)), trace=trace)
    return res


def sse_from_stats(out):
    """out: [128, NGROUPS*6] bn_stats rows -> sum(resid^2) over the shard."""
    s = out.astype(np.float64).reshape(128, NGROUPS, 2, 3)
    cnt, mean, cvar = s[..., 0], s[..., 1], s[..., 2]
    return (cvar + cnt * mean * mean).sum()


def kernel(data, W_pre, H_pre):
    res = run_cores(data, W_pre, H_pre, mode="fp8", trace=False)
    sse = np.float64(0.0)
    for r in res.results:
        sse += sse_from_stats(r["out"])
    return np.float32(sse / (F * T))
